# revision 18
# baseline (speedup 1.0000x reference)
"""Dense 2-layer 2-head GAT for Trainium2 (Bass/Tile), data-parallel over batch.

Each of the 8 NeuronCores processes one batch element (B=8). The per-head
attention score matrix s[i,j] = leakyrelu(hl_i + hr_j) is rank-1 structured,
so score tiles are generated on-chip (never materialized in DRAM):

  - hl broadcast across partitions comes from a single matmul with a
    column-replicated `a_l` stationary operand against hTb.
  - hr enters as a per-partition scalar: DVE tensor_scalar add (bf16) or
    fused into an ACT Prelu bias. The DVE/ACT split is a static balance
    knob (PATHA_COUNT).
  - leakyrelu uses AF.Prelu (parametric_relu), which lives in the same
    activation table set as Exp - no ACT table reloads anywhere. On the
    DVE path it is split as tensor_scalar mul (4x mode) + tensor_tensor
    max (2x) instead of the 1x-only fused scalar_tensor_tensor.
  - the diagonal mask is applied in place as a -1e30 stripe via GPSIMD
    affine_select on the diagonal-crossing tiles, so softmax needs no
    diagonal-correction epilogue.
  - exp on ACT writes bf16; p @ h and the denominator are bf16 matmuls
    (fp32 PSUM accumulate).
  - softmax runs without max-subtraction (scores bounded ~11, exp <= 5e4,
    fp32/bf16-safe; the unnormalized ratio is shift-invariant).

The two heads of a layer are issued interleaved (per score tile), so each
engine's in-order queue always holds independent work from the sibling head
- this fills the dependency bubbles of the score pipeline and keeps the PE
busy (HAM stays unthrottled). Both heads' softmax denominators share one
[4, IB] PSUM tile: head0's ones-stationary has columns [1,1,0,0] (rows 0-1),
head1's [0,0,1,1] (rows 2-3), accumulated in a single long group.

Everything stays in the transposed layout [feat_part, node_free] so each
layer's output feeds the next layer's matmul directly; only the initial x
load and final store transpose via the PE.
"""

import os
from contextlib import ExitStack

import numpy as np

import concourse.bass as bass
import concourse.mybir as mybir
import concourse.tile as tile
from concourse.alu_op_type import AluOpType
from concourse.masks import make_identity

F32 = mybir.dt.float32
F32R = mybir.dt.float32r
BF16 = mybir.dt.bfloat16
AF = mybir.ActivationFunctionType

N = 2048
F = 256
D = 128
P = 128
ALPHA = 0.2
NEG = -1.0e30
N_CORES = 8

# number of jc indices per half whose score tiles are generated entirely on
# ACT (Prelu-with-bias + Exp); the rest use DVE add+mul+max + ACT Exp.
PATHA_COUNT = 5


def build_nc(n=N):
    from concourse import bacc
    nc = bacc.Bacc("TRN2", target_bir_lowering=False, debug=False,
                   enable_asserts=False, num_devices=N_CORES)

    x_d = nc.declare_dram_parameter("x", [n, F], F32, isOutput=False)
    W_d, b_d, a_d = {}, {}, {}
    for l in (0, 1):
        for h in (0, 1):
            W_d[l, h] = nc.declare_dram_parameter(f"W_{l}_{h}", [F, D], F32, isOutput=False)
            b_d[l, h] = nc.declare_dram_parameter(f"b_{l}_{h}", [D], F32, isOutput=False)
            a_d[l, h] = nc.declare_dram_parameter(f"a_{l}_{h}", [2 * D, 1], F32, isOutput=False)
    out_d = nc.declare_dram_parameter("out", [n, F], F32, isOutput=True)

    NJ = n // P          # node chunks of 128 (partition dim of score tiles)
    IB = min(512, n)     # i-block width (one PSUM bank)
    NI = n // IB
    HW = 2 * IB          # half width (score tile free dim)
    NHALF = n // HW
    KH = HW // IB

    with tile.TileContext(nc) as tc, ExitStack() as ctx:
        const = ctx.enter_context(tc.tile_pool(name="const", bufs=1))
        persist = ctx.enter_context(tc.tile_pool(name="persist", bufs=1))
        headp = ctx.enter_context(tc.tile_pool(name="headp", bufs=2))
        ztp = ctx.enter_context(tc.tile_pool(name="ztp", bufs=6))
        lrp = ctx.enter_context(tc.tile_pool(name="lrp", bufs=6))
        up = ctx.enter_context(tc.tile_pool(name="up", bufs=10))
        epp = ctx.enter_context(tc.tile_pool(name="epp", bufs=3))
        smallp = ctx.enter_context(tc.tile_pool(name="smallp", bufs=4))
        ps_prep = ctx.enter_context(tc.tile_pool(name="ps_prep", bufs=2, space="PSUM"))
        ps_main = ctx.enter_context(tc.tile_pool(name="ps_main", bufs=2, space="PSUM"))
        ps_z = ctx.enter_context(tc.tile_pool(name="ps_z", bufs=1, space="PSUM"))

        # ---- constants ----
        I128 = const.tile([P, P], F32, tag="I128", name="I128")
        make_identity(nc, I128[:])
        I128b = const.tile([P, P], BF16, tag="I128b", name="I128b")
        nc.vector.tensor_copy(I128b[:], I128[:])
        ones_col_f = const.tile([P, 1], F32, tag="ones_col_f", name="ones_col_f")
        nc.vector.memset(ones_col_f[:], 1.0)
        # ones4[0]: cols [1,1,0,0] -> head0 denominator in rows 0-1;
        # ones4[1]: cols [0,0,1,1] -> head1 denominator in rows 2-3.
        ones4 = []
        for hi in range(2):
            o4f = const.tile([P, 4], F32, tag=f"o4f{hi}", name=f"o4f{hi}")
            nc.vector.memset(o4f[:], 0.0)
            nc.vector.memset(o4f[:, 2 * hi:2 * hi + 2], 1.0)
            o4 = const.tile([P, 4], BF16, tag=f"o4{hi}", name=f"o4{hi}")
            nc.vector.tensor_copy(o4[:], o4f[:])
            ones4.append(o4)
        # sel4[hi]: [4, P] selector stationary - row 2*hi is ones, other rows
        # zero - so matmul(sel4[hi], recip[4, IB]) broadcasts head hi's
        # reciprocal row across all 128 partitions with every AP at partition 0.
        sel4 = []
        for hi in range(2):
            s4f = const.tile([4, P], F32, tag=f"s4f{hi}", name=f"s4f{hi}")
            nc.gpsimd.memset(s4f[:], 0.0)
            # keep 0 where p != 2*hi, fill 1.0 on row p == 2*hi
            nc.gpsimd.affine_select(
                out=s4f[:], in_=s4f[:], compare_op=AluOpType.not_equal,
                fill=1.0, base=-2 * hi, pattern=[[0, P]], channel_multiplier=1)
            s4 = const.tile([4, P], BF16, tag=f"s4{hi}", name=f"s4{hi}")
            nc.vector.tensor_copy(s4[:], s4f[:])
            sel4.append(s4)

        # ---- parameters ----
        Wt, bt, Alt, art = {}, {}, {}, {}
        for l in (0, 1):
            for h in (0, 1):
                Wt[l, h] = []
                for c in range(2):
                    wf = smallp.tile([P, D], F32, tag="wload", name="wload")
                    nc.sync.dma_start(out=wf[:], in_=W_d[l, h][c * P:(c + 1) * P, :])
                    w = const.tile([P, D], F32R, tag=f"W{l}{h}{c}", name=f"W{l}{h}{c}")
                    nc.vector.tensor_copy(w[:], wf[:])
                    Wt[l, h].append(w)
                b = const.tile([P, 1], F32, tag=f"b{l}{h}", name=f"b{l}{h}")
                nc.sync.dma_start(
                    out=b[:], in_=b_d[l, h][:].rearrange("(p o) -> p o", o=1))
                bt[l, h] = b
                alf = smallp.tile([P, 1], F32, tag="alload", name="alload")
                nc.sync.dma_start(out=alf[:], in_=a_d[l, h][0:P, 0:1])
                Al = const.tile([P, P], BF16, tag=f"Al{l}{h}", name=f"Al{l}{h}")
                nc.vector.tensor_copy(Al[:], alf[:].to_broadcast([P, P]))
                Alt[l, h] = Al
                arf = smallp.tile([P, 1], F32, tag="arload", name="arload")
                nc.sync.dma_start(out=arf[:], in_=a_d[l, h][P:2 * P, 0:1])
                ar2 = const.tile([P, 2], BF16, tag=f"ar{l}{h}", name=f"ar{l}{h}")
                nc.vector.tensor_copy(ar2[:], arf[:].to_broadcast([P, 2]))
                art[l, h] = ar2

        # ---- load x and transpose to xT [2 x (P, n)] (f32r: feeds hT-mm) ----
        xT = [persist.tile([P, n], F32R, tag=f"xT{f}", name=f"xT{f}") for f in range(2)]
        for c in range(NJ):
            xc = smallp.tile([P, F], F32, tag="xload", name="xload")
            nc.sync.dma_start(out=xc[:], in_=x_d[c * P:(c + 1) * P, :])
            for f in range(2):
                tp = ps_prep.tile([P, IB], F32, tag="prep", name="prep")
                nc.tensor.transpose(tp[:, 0:P], xc[:, f * P:(f + 1) * P], I128[:])
                if (c + f) % 2 == 0:
                    nc.vector.tensor_copy(xT[f][:, c * P:(c + 1) * P], tp[:, 0:P])
                else:
                    nc.scalar.activation(xT[f][:, c * P:(c + 1) * P], tp[:, 0:P], AF.Copy)

        X1T = [persist.tile([P, n], F32R, tag=f"X1T{f}", name=f"X1T{f}") for f in range(2)]
        X2T = [persist.tile([P, n], F32, tag=f"X2T{f}", name=f"X2T{f}") for f in range(2)]

        def gat_layer(XT, pars, OUTS):
            # ---- per-head prep, issued interleaved ----
            hd = [dict(), dict()]
            for hi in range(2):
                hd[hi]["hT"] = headp.tile([P, n], F32R, tag="hT", name=f"hT{hi}")
                hd[hi]["hTf"] = hd[hi]["hT"][:].bitcast(F32)
                hd[hi]["hTb"] = headp.tile([P, n], BF16, tag="hTb", name=f"hTb{hi}")
                hd[hi]["hlb"] = headp.tile([P, n], BF16, tag="hlb", name=f"hlb{hi}")
                hd[hi]["hrcf"] = headp.tile([P, NJ], F32, tag="hrcf", name=f"hrcf{hi}")
                hd[hi]["hs"] = []
            for ib in range(NI):
                sl = slice(ib * IB, (ib + 1) * IB)
                for hi, (Wc, b, Al, ar2) in enumerate(pars):
                    ps = ps_prep.tile([P, IB], F32, tag="prep", name="prep")
                    nc.tensor.matmul(ps[:], Wc[0][:], XT[0][:, sl], start=True, stop=False)
                    nc.tensor.matmul(ps[:], Wc[1][:], XT[1][:, sl], start=False, stop=True)
                    nc.vector.tensor_scalar_add(hd[hi]["hT"][:, sl], ps[:], b[:])
            for hi in range(2):
                nc.vector.tensor_copy(hd[hi]["hTb"][:], hd[hi]["hTf"][:])
            # h chunks [node_part, d_free] bf16 via PE transpose
            for jc in range(NJ):
                for hi in range(2):
                    tp = ps_prep.tile([P, IB], BF16, tag="prep", name="prep")
                    nc.tensor.transpose(tp[:, 0:P], hd[hi]["hTb"][:, jc * P:(jc + 1) * P], I128b[:])
                    hj = headp.tile([P, P], BF16, tag=f"h{jc}", name=f"h{jc}_{hi}")
                    if (jc + hi) % 2 == 0:
                        nc.vector.tensor_copy(hj[:], tp[:, 0:P])
                    else:
                        nc.scalar.activation(hj[:], tp[:, 0:P], AF.Copy)
                    hd[hi]["hs"].append(hj)
            # hl broadcast across partitions + hr column layout
            for ib in range(NI):
                sl = slice(ib * IB, (ib + 1) * IB)
                for hi, (Wc, b, Al, ar2) in enumerate(pars):
                    ps = ps_prep.tile([P, IB], F32, tag="prep", name="prep")
                    nc.tensor.matmul(ps[:], Al[:], hd[hi]["hTb"][:, sl], start=True, stop=True)
                    nc.vector.tensor_copy(hd[hi]["hlb"][:, sl], ps[:])
            for hi, (Wc, b, Al, ar2) in enumerate(pars):
                psr = ps_prep.tile([P, IB], F32, tag="prep", name="prep")
                for jc in range(NJ):
                    nc.tensor.matmul(psr[:, 2 * jc:2 * jc + 2],
                                     hd[hi]["hTb"][:, jc * P:(jc + 1) * P],
                                     ar2[:], start=True, stop=True)
                pair = psr[:, 0:2 * NJ].rearrange("p (c t) -> p c t", t=2)
                nc.vector.tensor_copy(hd[hi]["hrcf"][:], pair[:, :, 0])

            # ---- score loops, the two heads interleaved per tile ----
            # The previous half's epilogue is deferred and emitted a few
            # tiles into the next half's loop: the next half's DVE/ACT
            # score-gen fills the queue ahead of the epilogue ops (which
            # block on the PE finishing the previous half's matmul tail),
            # avoiding head-of-line stalls. The deferred tiles' matmuls are
            # flushed AFTER the epilogue so its rb matmuls stay ahead of
            # them in the PE queue (else PE deadlocks on the bank ring).
            def make_epilogue(half, zp, oaccs, hTfs):
                def emit():
                    for k in range(KH):
                        ib = half * KH + k
                        isl = slice(ib * IB, (ib + 1) * IB)
                        recip_f = smallp.tile([4, IB], F32, tag="recip_f",
                                              name="recip_f", bufs=2)
                        nc.vector.reciprocal_approx_fast(recip_f[:], zp[k][:])
                        recip = smallp.tile([4, IB], BF16, tag="recip",
                                            name="recip", bufs=2)
                        nc.vector.tensor_copy(recip[:], recip_f[:])
                        for hi in range(2):
                            rb = ps_prep.tile([P, IB], F32, tag="prep", name="prep")
                            nc.tensor.matmul(rb[:], sel4[hi][:], recip[:],
                                             start=True, stop=True)
                            rbs = epp.tile([P, IB], F32, tag="rbs", name="rbs")
                            nc.vector.tensor_copy(rbs[:], rb[:])
                            v = epp.tile([P, IB], F32, tag="v", name="v")
                            nc.vector.tensor_tensor(v[:], oaccs[hi][k][:], rbs[:],
                                                    AluOpType.mult)
                            v2 = epp.tile([P, IB], F32, tag="v2", name="v2")
                            nc.vector.tensor_tensor(v2[:], v[:], hTfs[hi][:, isl],
                                                    AluOpType.add)
                            # elu(v2) = relu(v2) + exp(-relu(-v2)) - 1
                            r1 = epp.tile([P, IB], F32, tag="r1", name="r1")
                            nc.scalar.activation(r1[:], v2[:], AF.Relu, scale=-1.0)
                            r3 = epp.tile([P, IB], F32, tag="r3", name="r3")
                            nc.vector.tensor_scalar(r3[:], v2[:], 0.0, None,
                                                    AluOpType.max)
                            r2 = epp.tile([P, IB], F32, tag="r2", name="r2")
                            nc.scalar.activation(r2[:], r1[:], AF.Exp, scale=-1.0)
                            nc.vector.scalar_tensor_tensor(
                                OUTS[hi][:, isl], in0=r2[:], scalar=-1.0, in1=r3[:],
                                op0=AluOpType.add, op1=AluOpType.add)
                return emit

            PIPE_TILES = 6
            pending_epi = None
            for half in range(NHALF):
                hsl = slice(half * HW, (half + 1) * HW)
                for hi in range(2):
                    hd[hi]["oacc"] = [
                        ps_main.tile([P, IB], F32, tag=f"oacc{k}", name=f"oacc{k}_{hi}")
                        for k in range(KH)]
                # both heads' denominators share one [4, IB] tile per k
                zp = [ps_z.tile([4, IB], F32, tag=f"zp{k}", name=f"zp{k}")
                      for k in range(KH)]
                dlo, dhi = half * (NJ // NHALF), (half + 1) * (NJ // NHALF)
                patha = set()
                step = max(1, (NJ - (dhi - dlo)) // max(1, PATHA_COUNT))
                cnt = 0
                for jc in range(NJ):
                    if not (dlo <= jc < dhi) and cnt < PATHA_COUNT and (jc % step) == step - 1:
                        patha.add(jc)
                        cnt += 1
                deferred = []
                npipe = PIPE_TILES if pending_epi is not None else 0
                tile_idx = 0
                for jc in range(NJ):
                    diag = dlo <= jc < dhi
                    for hi in range(2):
                        H = hd[hi]
                        u = up.tile([P, HW], BF16, tag="u", name="u")
                        if jc in patha:
                            lr = lrp.tile([P, HW], BF16, tag="lr", name="lr")
                            nc.scalar.activation(lr[:], H["hlb"][:, hsl], AF.Prelu,
                                                 bias=H["hrcf"][:, jc:jc + 1], alpha=ALPHA)
                            nc.scalar.activation(u[:], lr[:], AF.Exp)
                        else:
                            zt = ztp.tile([P, HW], BF16, tag="zt", name="zt")
                            nc.vector.tensor_scalar_add(zt[:], H["hlb"][:, hsl],
                                                        H["hrcf"][:, jc:jc + 1])
                            za = ztp.tile([P, HW], BF16, tag="za", name="za")
                            nc.vector.tensor_scalar_mul(za[:], zt[:], ALPHA)
                            nc.vector.tensor_tensor(zt[:], zt[:], za[:],
                                                    AluOpType.max)
                            if diag:
                                off = jc * P - half * HW
                                nc.gpsimd.affine_select(
                                    out=zt[:, off:off + P], in_=zt[:, off:off + P],
                                    compare_op=AluOpType.not_equal,
                                    fill=NEG, base=0, pattern=[[-1, P]],
                                    channel_multiplier=1)
                            nc.scalar.activation(u[:], zt[:], AF.Exp)

                        def mms(H=H, u=u, jc=jc, hi=hi):
                            for k in range(KH):
                                nc.tensor.matmul(H["oacc"][k][:], H["hs"][jc][:],
                                                 u[:, k * IB:(k + 1) * IB],
                                                 start=(jc == 0), stop=(jc == NJ - 1))
                            for k in range(KH):
                                nc.tensor.matmul(zp[k][:], ones4[hi][:],
                                                 u[:, k * IB:(k + 1) * IB],
                                                 start=(jc == 0 and hi == 0),
                                                 stop=(jc == NJ - 1 and hi == 1))
                        if tile_idx < npipe:
                            deferred.append(mms)
                        else:
                            if tile_idx == npipe and pending_epi is not None:
                                pending_epi()
                                pending_epi = None
                                for m in deferred:
                                    m()
                                deferred = []
                            mms()
                        tile_idx += 1
                pending_epi = make_epilogue(
                    half, zp, [hd[hi]["oacc"] for hi in range(2)],
                    [hd[hi]["hTf"] for hi in range(2)])
            pending_epi()

        gat_layer(xT, [(Wt[0, 0], bt[0, 0], Alt[0, 0], art[0, 0]),
                       (Wt[0, 1], bt[0, 1], Alt[0, 1], art[0, 1])], X1T)
        gat_layer(X1T, [(Wt[1, 0], bt[1, 0], Alt[1, 0], art[1, 0]),
                        (Wt[1, 1], bt[1, 1], Alt[1, 1], art[1, 1])], X2T)

        # ---- transpose X2T back and store ----
        for c in range(NJ):
            ob = smallp.tile([P, F], F32, tag="ob", name="ob")
            for f in range(2):
                tp = ps_prep.tile([P, IB], F32, tag="prep", name="prep")
                nc.tensor.transpose(tp[:, 0:P], X2T[f][:, c * P:(c + 1) * P], I128[:])
                if (c + f) % 2 == 0:
                    nc.vector.tensor_copy(ob[:, f * P:(f + 1) * P], tp[:, 0:P])
                else:
                    nc.scalar.activation(ob[:, f * P:(f + 1) * P], tp[:, 0:P], AF.Copy)
            nc.sync.dma_start(out=out_d[c * P:(c + 1) * P, :], in_=ob[:])

    nc.compile()
    return nc


_CACHE = {}
LAST_RESULTS = None


def kernel(**inputs):
    global LAST_RESULTS
    from concourse.bass_utils import run_bass_kernel_spmd

    x = np.ascontiguousarray(np.asarray(inputs["x"], dtype=np.float32))
    B = x.shape[0]
    assert B == N_CORES and x.shape[1] == N and x.shape[2] == F

    if "nc" not in _CACHE:
        _CACHE["nc"] = build_nc()
    nc = _CACHE["nc"]

    base = {}
    for l in (0, 1):
        for h in (0, 1):
            base[f"W_{l}_{h}"] = np.ascontiguousarray(
                np.asarray(inputs[f"W_{l}_{h}"], dtype=np.float32))
            base[f"b_{l}_{h}"] = np.ascontiguousarray(
                np.asarray(inputs[f"b_{l}_{h}"], dtype=np.float32))
            base[f"a_{l}_{h}"] = np.ascontiguousarray(
                np.asarray(inputs[f"a_{l}_{h}"], dtype=np.float32))

    in_maps = [dict(base, x=np.ascontiguousarray(x[i])) for i in range(B)]
    res = run_bass_kernel_spmd(nc, in_maps, list(range(N_CORES)),
                               trace=bool(os.environ.get("BASS_TRACE")))
    LAST_RESULTS = res
    out = np.stack([res.results[i]["out"] for i in range(B)], axis=0)
    return out.astype(np.float32)


# revision 19
# speedup vs baseline: 1.0000x; 1.0000x over previous
"""Dense 2-layer 2-head GAT for Trainium2 (Bass/Tile), data-parallel over batch.

Each of the 8 NeuronCores processes one batch element (B=8). The per-head
attention score matrix s[i,j] = leakyrelu(hl_i + hr_j) is rank-1 structured,
so score tiles are generated on-chip (never materialized in DRAM):

  - hl broadcast across partitions comes from a single matmul with a
    column-replicated `a_l` stationary operand against hTb.
  - hr enters as a per-partition scalar: DVE tensor_scalar add (bf16) or
    fused into an ACT Prelu bias. The DVE/ACT split is a static balance
    knob (PATHA_COUNT).
  - leakyrelu uses AF.Prelu (parametric_relu), which lives in the same
    activation table set as Exp - no ACT table reloads anywhere. On the
    DVE path it is split as tensor_scalar mul (4x mode) + tensor_tensor
    max (2x) instead of the 1x-only fused scalar_tensor_tensor.
  - the diagonal mask is applied in place as a -1e30 stripe via GPSIMD
    affine_select on the diagonal-crossing tiles, so softmax needs no
    diagonal-correction epilogue.
  - exp on ACT writes bf16; p @ h and the denominator are bf16 matmuls
    (fp32 PSUM accumulate).
  - softmax runs without max-subtraction (scores bounded ~11, exp <= 5e4,
    fp32/bf16-safe; the unnormalized ratio is shift-invariant).

The two heads of a layer are issued interleaved (per score tile), so each
engine's in-order queue always holds independent work from the sibling head
- this fills the dependency bubbles of the score pipeline and keeps the PE
busy (HAM stays unthrottled). Both heads' softmax denominators share one
[4, IB] PSUM tile: head0's ones-stationary has columns [1,1,0,0] (rows 0-1),
head1's [0,0,1,1] (rows 2-3), accumulated in a single long group.

Everything stays in the transposed layout [feat_part, node_free] so each
layer's output feeds the next layer's matmul directly; only the initial x
load and final store transpose via the PE.
"""

import os
from contextlib import ExitStack

import numpy as np

import concourse.bass as bass
import concourse.mybir as mybir
import concourse.tile as tile
from concourse.alu_op_type import AluOpType
from concourse.masks import make_identity

F32 = mybir.dt.float32
F32R = mybir.dt.float32r
BF16 = mybir.dt.bfloat16
AF = mybir.ActivationFunctionType

N = 2048
F = 256
D = 128
P = 128
ALPHA = 0.2
NEG = -1.0e30
N_CORES = 8

# number of jc indices per half whose score tiles are generated entirely on
# ACT (Prelu-with-bias + Exp); the rest use DVE add+mul+max + ACT Exp.
PATHA_COUNT = 5


def build_nc(n=N):
    from concourse import bacc
    nc = bacc.Bacc("TRN2", target_bir_lowering=False, debug=False,
                   enable_asserts=False, num_devices=N_CORES)

    x_d = nc.declare_dram_parameter("x", [n, F], F32, isOutput=False)
    W_d, b_d, a_d = {}, {}, {}
    for l in (0, 1):
        for h in (0, 1):
            W_d[l, h] = nc.declare_dram_parameter(f"W_{l}_{h}", [F, D], F32, isOutput=False)
            b_d[l, h] = nc.declare_dram_parameter(f"b_{l}_{h}", [D], F32, isOutput=False)
            a_d[l, h] = nc.declare_dram_parameter(f"a_{l}_{h}", [2 * D, 1], F32, isOutput=False)
    out_d = nc.declare_dram_parameter("out", [n, F], F32, isOutput=True)

    NJ = n // P          # node chunks of 128 (partition dim of score tiles)
    IB = min(512, n)     # i-block width (one PSUM bank)
    NI = n // IB
    HW = 2 * IB          # half width (score tile free dim)
    NHALF = n // HW
    KH = HW // IB

    with tile.TileContext(nc) as tc, ExitStack() as ctx:
        const = ctx.enter_context(tc.tile_pool(name="const", bufs=1))
        persist = ctx.enter_context(tc.tile_pool(name="persist", bufs=1))
        headp = ctx.enter_context(tc.tile_pool(name="headp", bufs=2))
        ztp = ctx.enter_context(tc.tile_pool(name="ztp", bufs=8))
        zap = ctx.enter_context(tc.tile_pool(name="zap", bufs=4))
        lrp = ctx.enter_context(tc.tile_pool(name="lrp", bufs=4))
        up = ctx.enter_context(tc.tile_pool(name="up", bufs=16))
        epp = ctx.enter_context(tc.tile_pool(name="epp", bufs=2))
        smallp = ctx.enter_context(tc.tile_pool(name="smallp", bufs=4))
        ps_prep = ctx.enter_context(tc.tile_pool(name="ps_prep", bufs=2, space="PSUM"))
        ps_main = ctx.enter_context(tc.tile_pool(name="ps_main", bufs=2, space="PSUM"))
        ps_z = ctx.enter_context(tc.tile_pool(name="ps_z", bufs=1, space="PSUM"))

        # ---- constants ----
        I128 = const.tile([P, P], F32, tag="I128", name="I128")
        make_identity(nc, I128[:])
        I128b = const.tile([P, P], BF16, tag="I128b", name="I128b")
        nc.vector.tensor_copy(I128b[:], I128[:])
        ones_col_f = const.tile([P, 1], F32, tag="ones_col_f", name="ones_col_f")
        nc.vector.memset(ones_col_f[:], 1.0)
        # ones4[0]: cols [1,1,0,0] -> head0 denominator in rows 0-1;
        # ones4[1]: cols [0,0,1,1] -> head1 denominator in rows 2-3.
        ones4 = []
        for hi in range(2):
            o4f = const.tile([P, 4], F32, tag=f"o4f{hi}", name=f"o4f{hi}")
            nc.vector.memset(o4f[:], 0.0)
            nc.vector.memset(o4f[:, 2 * hi:2 * hi + 2], 1.0)
            o4 = const.tile([P, 4], BF16, tag=f"o4{hi}", name=f"o4{hi}")
            nc.vector.tensor_copy(o4[:], o4f[:])
            ones4.append(o4)
        # sel4[hi]: [4, P] selector stationary - row 2*hi is ones, other rows
        # zero - so matmul(sel4[hi], recip[4, IB]) broadcasts head hi's
        # reciprocal row across all 128 partitions with every AP at partition 0.
        sel4 = []
        for hi in range(2):
            s4f = const.tile([4, P], F32, tag=f"s4f{hi}", name=f"s4f{hi}")
            nc.gpsimd.memset(s4f[:], 0.0)
            # keep 0 where p != 2*hi, fill 1.0 on row p == 2*hi
            nc.gpsimd.affine_select(
                out=s4f[:], in_=s4f[:], compare_op=AluOpType.not_equal,
                fill=1.0, base=-2 * hi, pattern=[[0, P]], channel_multiplier=1)
            s4 = const.tile([4, P], BF16, tag=f"s4{hi}", name=f"s4{hi}")
            nc.vector.tensor_copy(s4[:], s4f[:])
            sel4.append(s4)

        # ---- parameters ----
        Wt, bt, Alt, art = {}, {}, {}, {}
        for l in (0, 1):
            for h in (0, 1):
                Wt[l, h] = []
                for c in range(2):
                    wf = smallp.tile([P, D], F32, tag="wload", name="wload")
                    nc.sync.dma_start(out=wf[:], in_=W_d[l, h][c * P:(c + 1) * P, :])
                    w = const.tile([P, D], F32R, tag=f"W{l}{h}{c}", name=f"W{l}{h}{c}")
                    nc.vector.tensor_copy(w[:], wf[:])
                    Wt[l, h].append(w)
                b = const.tile([P, 1], F32, tag=f"b{l}{h}", name=f"b{l}{h}")
                nc.sync.dma_start(
                    out=b[:], in_=b_d[l, h][:].rearrange("(p o) -> p o", o=1))
                bt[l, h] = b
                alf = smallp.tile([P, 1], F32, tag="alload", name="alload")
                nc.sync.dma_start(out=alf[:], in_=a_d[l, h][0:P, 0:1])
                Al = const.tile([P, P], BF16, tag=f"Al{l}{h}", name=f"Al{l}{h}")
                nc.vector.tensor_copy(Al[:], alf[:].to_broadcast([P, P]))
                Alt[l, h] = Al
                arf = smallp.tile([P, 1], F32, tag="arload", name="arload")
                nc.sync.dma_start(out=arf[:], in_=a_d[l, h][P:2 * P, 0:1])
                ar2 = const.tile([P, 2], BF16, tag=f"ar{l}{h}", name=f"ar{l}{h}")
                nc.vector.tensor_copy(ar2[:], arf[:].to_broadcast([P, 2]))
                art[l, h] = ar2

        # ---- load x and transpose to xT [2 x (P, n)] (f32r: feeds hT-mm) ----
        xT = [persist.tile([P, n], F32R, tag=f"xT{f}", name=f"xT{f}") for f in range(2)]
        for c in range(NJ):
            xc = smallp.tile([P, F], F32, tag="xload", name="xload")
            nc.sync.dma_start(out=xc[:], in_=x_d[c * P:(c + 1) * P, :])
            for f in range(2):
                tp = ps_prep.tile([P, IB], F32, tag="prep", name="prep")
                nc.tensor.transpose(tp[:, 0:P], xc[:, f * P:(f + 1) * P], I128[:])
                if (c + f) % 2 == 0:
                    nc.vector.tensor_copy(xT[f][:, c * P:(c + 1) * P], tp[:, 0:P])
                else:
                    nc.scalar.activation(xT[f][:, c * P:(c + 1) * P], tp[:, 0:P], AF.Copy)

        X1T = [persist.tile([P, n], F32R, tag=f"X1T{f}", name=f"X1T{f}") for f in range(2)]
        X2T = [persist.tile([P, n], F32, tag=f"X2T{f}", name=f"X2T{f}") for f in range(2)]

        def gat_layer(XT, pars, OUTS):
            # ---- per-head prep, issued interleaved ----
            hd = [dict(), dict()]
            for hi in range(2):
                hd[hi]["hT"] = headp.tile([P, n], F32R, tag="hT", name=f"hT{hi}")
                hd[hi]["hTf"] = hd[hi]["hT"][:].bitcast(F32)
                hd[hi]["hTb"] = headp.tile([P, n], BF16, tag="hTb", name=f"hTb{hi}")
                hd[hi]["hlb"] = headp.tile([P, n], BF16, tag="hlb", name=f"hlb{hi}")
                hd[hi]["hrcf"] = headp.tile([P, NJ], F32, tag="hrcf", name=f"hrcf{hi}")
                hd[hi]["hs"] = []
            for ib in range(NI):
                sl = slice(ib * IB, (ib + 1) * IB)
                for hi, (Wc, b, Al, ar2) in enumerate(pars):
                    ps = ps_prep.tile([P, IB], F32, tag="prep", name="prep")
                    nc.tensor.matmul(ps[:], Wc[0][:], XT[0][:, sl], start=True, stop=False)
                    nc.tensor.matmul(ps[:], Wc[1][:], XT[1][:, sl], start=False, stop=True)
                    nc.vector.tensor_scalar_add(hd[hi]["hT"][:, sl], ps[:], b[:])
            for hi in range(2):
                nc.vector.tensor_copy(hd[hi]["hTb"][:], hd[hi]["hTf"][:])
            # h chunks [node_part, d_free] bf16 via PE transpose
            for jc in range(NJ):
                for hi in range(2):
                    tp = ps_prep.tile([P, IB], BF16, tag="prep", name="prep")
                    nc.tensor.transpose(tp[:, 0:P], hd[hi]["hTb"][:, jc * P:(jc + 1) * P], I128b[:])
                    hj = headp.tile([P, P], BF16, tag=f"h{jc}", name=f"h{jc}_{hi}")
                    if (jc + hi) % 2 == 0:
                        nc.vector.tensor_copy(hj[:], tp[:, 0:P])
                    else:
                        nc.scalar.activation(hj[:], tp[:, 0:P], AF.Copy)
                    hd[hi]["hs"].append(hj)
            # hl broadcast across partitions + hr column layout
            for ib in range(NI):
                sl = slice(ib * IB, (ib + 1) * IB)
                for hi, (Wc, b, Al, ar2) in enumerate(pars):
                    ps = ps_prep.tile([P, IB], F32, tag="prep", name="prep")
                    nc.tensor.matmul(ps[:], Al[:], hd[hi]["hTb"][:, sl], start=True, stop=True)
                    nc.vector.tensor_copy(hd[hi]["hlb"][:, sl], ps[:])
            for hi, (Wc, b, Al, ar2) in enumerate(pars):
                psr = ps_prep.tile([P, IB], F32, tag="prep", name="prep")
                for jc in range(NJ):
                    nc.tensor.matmul(psr[:, 2 * jc:2 * jc + 2],
                                     hd[hi]["hTb"][:, jc * P:(jc + 1) * P],
                                     ar2[:], start=True, stop=True)
                pair = psr[:, 0:2 * NJ].rearrange("p (c t) -> p c t", t=2)
                nc.vector.tensor_copy(hd[hi]["hrcf"][:], pair[:, :, 0])

            # ---- score loops, the two heads interleaved per tile ----
            # The previous half's epilogue is deferred and emitted a few
            # tiles into the next half's loop: the next half's DVE/ACT
            # score-gen fills the queue ahead of the epilogue ops (which
            # block on the PE finishing the previous half's matmul tail),
            # avoiding head-of-line stalls. The deferred tiles' matmuls are
            # flushed AFTER the epilogue so its rb matmuls stay ahead of
            # them in the PE queue (else PE deadlocks on the bank ring).
            def make_epilogue(half, zp, oaccs, hTfs):
                def emit():
                    for k in range(KH):
                        ib = half * KH + k
                        isl = slice(ib * IB, (ib + 1) * IB)
                        recip_f = smallp.tile([4, IB], F32, tag="recip_f",
                                              name="recip_f", bufs=2)
                        nc.vector.reciprocal_approx_fast(recip_f[:], zp[k][:])
                        recip = smallp.tile([4, IB], BF16, tag="recip",
                                            name="recip", bufs=2)
                        nc.vector.tensor_copy(recip[:], recip_f[:])
                        for hi in range(2):
                            rb = ps_prep.tile([P, IB], F32, tag="prep", name="prep")
                            nc.tensor.matmul(rb[:], sel4[hi][:], recip[:],
                                             start=True, stop=True)
                            rbs = epp.tile([P, IB], F32, tag="rbs", name="rbs")
                            nc.vector.tensor_copy(rbs[:], rb[:])
                            v = epp.tile([P, IB], F32, tag="v", name="v")
                            nc.vector.tensor_tensor(v[:], oaccs[hi][k][:], rbs[:],
                                                    AluOpType.mult)
                            v2 = epp.tile([P, IB], F32, tag="v2", name="v2")
                            nc.vector.tensor_tensor(v2[:], v[:], hTfs[hi][:, isl],
                                                    AluOpType.add)
                            # elu(v2) = relu(v2) + exp(-relu(-v2)) - 1
                            r1 = epp.tile([P, IB], F32, tag="r1", name="r1")
                            nc.scalar.activation(r1[:], v2[:], AF.Relu, scale=-1.0)
                            r3 = epp.tile([P, IB], F32, tag="r3", name="r3")
                            nc.vector.tensor_scalar(r3[:], v2[:], 0.0, None,
                                                    AluOpType.max)
                            r2 = epp.tile([P, IB], F32, tag="r2", name="r2")
                            nc.scalar.activation(r2[:], r1[:], AF.Exp, scale=-1.0)
                            nc.vector.scalar_tensor_tensor(
                                OUTS[hi][:, isl], in0=r2[:], scalar=-1.0, in1=r3[:],
                                op0=AluOpType.add, op1=AluOpType.add)
                return emit

            PIPE_TILES = 6
            pending_epi = None
            for half in range(NHALF):
                hsl = slice(half * HW, (half + 1) * HW)
                for hi in range(2):
                    hd[hi]["oacc"] = [
                        ps_main.tile([P, IB], F32, tag=f"oacc{k}", name=f"oacc{k}_{hi}")
                        for k in range(KH)]
                # both heads' denominators share one [4, IB] tile per k
                zp = [ps_z.tile([4, IB], F32, tag=f"zp{k}", name=f"zp{k}")
                      for k in range(KH)]
                dlo, dhi = half * (NJ // NHALF), (half + 1) * (NJ // NHALF)
                patha = set()
                step = max(1, (NJ - (dhi - dlo)) // max(1, PATHA_COUNT))
                cnt = 0
                for jc in range(NJ):
                    if not (dlo <= jc < dhi) and cnt < PATHA_COUNT and (jc % step) == step - 1:
                        patha.add(jc)
                        cnt += 1
                deferred = []
                npipe = PIPE_TILES if pending_epi is not None else 0
                tile_idx = 0
                for jc in range(NJ):
                    diag = dlo <= jc < dhi
                    for hi in range(2):
                        H = hd[hi]
                        u = up.tile([P, HW], BF16, tag="u", name="u")
                        if jc in patha:
                            lr = lrp.tile([P, HW], BF16, tag="lr", name="lr")
                            nc.scalar.activation(lr[:], H["hlb"][:, hsl], AF.Prelu,
                                                 bias=H["hrcf"][:, jc:jc + 1], alpha=ALPHA)
                            nc.scalar.activation(u[:], lr[:], AF.Exp)
                        else:
                            zt = ztp.tile([P, HW], BF16, tag="zt", name="zt")
                            nc.vector.tensor_scalar_add(zt[:], H["hlb"][:, hsl],
                                                        H["hrcf"][:, jc:jc + 1])
                            za = zap.tile([P, HW], BF16, tag="za", name="za")
                            nc.vector.tensor_scalar_mul(za[:], zt[:], ALPHA)
                            nc.vector.tensor_tensor(zt[:], zt[:], za[:],
                                                    AluOpType.max)
                            if diag:
                                off = jc * P - half * HW
                                nc.gpsimd.affine_select(
                                    out=zt[:, off:off + P], in_=zt[:, off:off + P],
                                    compare_op=AluOpType.not_equal,
                                    fill=NEG, base=0, pattern=[[-1, P]],
                                    channel_multiplier=1)
                            nc.scalar.activation(u[:], zt[:], AF.Exp)

                        def mms(H=H, u=u, jc=jc, hi=hi):
                            for k in range(KH):
                                nc.tensor.matmul(H["oacc"][k][:], H["hs"][jc][:],
                                                 u[:, k * IB:(k + 1) * IB],
                                                 start=(jc == 0), stop=(jc == NJ - 1))
                            for k in range(KH):
                                nc.tensor.matmul(zp[k][:], ones4[hi][:],
                                                 u[:, k * IB:(k + 1) * IB],
                                                 start=(jc == 0 and hi == 0),
                                                 stop=(jc == NJ - 1 and hi == 1))
                        if tile_idx < npipe:
                            deferred.append(mms)
                        else:
                            if tile_idx == npipe and pending_epi is not None:
                                pending_epi()
                                pending_epi = None
                                for m in deferred:
                                    m()
                                deferred = []
                            mms()
                        tile_idx += 1
                pending_epi = make_epilogue(
                    half, zp, [hd[hi]["oacc"] for hi in range(2)],
                    [hd[hi]["hTf"] for hi in range(2)])
            pending_epi()

        gat_layer(xT, [(Wt[0, 0], bt[0, 0], Alt[0, 0], art[0, 0]),
                       (Wt[0, 1], bt[0, 1], Alt[0, 1], art[0, 1])], X1T)
        gat_layer(X1T, [(Wt[1, 0], bt[1, 0], Alt[1, 0], art[1, 0]),
                        (Wt[1, 1], bt[1, 1], Alt[1, 1], art[1, 1])], X2T)

        # ---- transpose X2T back and store ----
        for c in range(NJ):
            ob = smallp.tile([P, F], F32, tag="ob", name="ob")
            for f in range(2):
                tp = ps_prep.tile([P, IB], F32, tag="prep", name="prep")
                nc.tensor.transpose(tp[:, 0:P], X2T[f][:, c * P:(c + 1) * P], I128[:])
                if (c + f) % 2 == 0:
                    nc.vector.tensor_copy(ob[:, f * P:(f + 1) * P], tp[:, 0:P])
                else:
                    nc.scalar.activation(ob[:, f * P:(f + 1) * P], tp[:, 0:P], AF.Copy)
            nc.sync.dma_start(out=out_d[c * P:(c + 1) * P, :], in_=ob[:])

    nc.compile()
    return nc


_CACHE = {}
LAST_RESULTS = None


def kernel(**inputs):
    global LAST_RESULTS
    from concourse.bass_utils import run_bass_kernel_spmd

    x = np.ascontiguousarray(np.asarray(inputs["x"], dtype=np.float32))
    B = x.shape[0]
    assert B == N_CORES and x.shape[1] == N and x.shape[2] == F

    if "nc" not in _CACHE:
        _CACHE["nc"] = build_nc()
    nc = _CACHE["nc"]

    base = {}
    for l in (0, 1):
        for h in (0, 1):
            base[f"W_{l}_{h}"] = np.ascontiguousarray(
                np.asarray(inputs[f"W_{l}_{h}"], dtype=np.float32))
            base[f"b_{l}_{h}"] = np.ascontiguousarray(
                np.asarray(inputs[f"b_{l}_{h}"], dtype=np.float32))
            base[f"a_{l}_{h}"] = np.ascontiguousarray(
                np.asarray(inputs[f"a_{l}_{h}"], dtype=np.float32))

    in_maps = [dict(base, x=np.ascontiguousarray(x[i])) for i in range(B)]
    res = run_bass_kernel_spmd(nc, in_maps, list(range(N_CORES)),
                               trace=bool(os.environ.get("BASS_TRACE")))
    LAST_RESULTS = res
    out = np.stack([res.results[i]["out"] for i in range(B)], axis=0)
    return out.astype(np.float32)


# revision 20
# speedup vs baseline: 1.0651x; 1.0650x over previous
"""Dense 2-layer 2-head GAT for Trainium2 (Bass/Tile), data-parallel over batch.

Each of the 8 NeuronCores processes one batch element (B=8). The per-head
attention score matrix s[i,j] = leakyrelu(hl_i + hr_j) is rank-1 structured,
so score tiles are generated on-chip (never materialized in DRAM):

  - hl broadcast across partitions comes from a single matmul with a
    column-replicated `a_l` stationary operand against hTb.
  - hr enters as a per-partition scalar: DVE tensor_scalar add (bf16) or
    fused into an ACT Prelu bias. The DVE/ACT split is a static balance
    knob (PATHA_COUNT).
  - leakyrelu uses AF.Prelu (parametric_relu), which lives in the same
    activation table set as Exp - no ACT table reloads anywhere. On the
    DVE path it is split as tensor_scalar mul (4x mode) + tensor_tensor
    max (2x) instead of the 1x-only fused scalar_tensor_tensor.
  - the diagonal mask is applied in place as a -1e30 stripe via GPSIMD
    affine_select on the diagonal-crossing tiles, so softmax needs no
    diagonal-correction epilogue.
  - exp on ACT writes bf16; p @ h and the denominator are bf16 matmuls
    (fp32 PSUM accumulate).
  - softmax runs without max-subtraction (scores bounded ~11, exp <= 5e4,
    fp32/bf16-safe; the unnormalized ratio is shift-invariant).

The two heads of a layer are issued interleaved (per score tile), so each
engine's in-order queue always holds independent work from the sibling head
- this fills the dependency bubbles of the score pipeline and keeps the PE
busy (HAM stays unthrottled). Both heads' softmax denominators share one
[4, IB] PSUM tile: head0's ones-stationary has columns [1,1,0,0] (rows 0-1),
head1's [0,0,1,1] (rows 2-3), accumulated in a single long group.

Everything stays in the transposed layout [feat_part, node_free] so each
layer's output feeds the next layer's matmul directly; only the initial x
load and final store transpose via the PE.
"""

import os
from contextlib import ExitStack

import numpy as np

import concourse.bass as bass
import concourse.mybir as mybir
import concourse.tile as tile
from concourse.alu_op_type import AluOpType
from concourse.masks import make_identity

F32 = mybir.dt.float32
F32R = mybir.dt.float32r
BF16 = mybir.dt.bfloat16
AF = mybir.ActivationFunctionType

N = 2048
F = 256
D = 128
P = 128
ALPHA = 0.2
NEG = -1.0e30
N_CORES = 8

# number of jc indices per half whose score tiles are generated entirely on
# ACT (Prelu-with-bias + Exp); the rest use DVE add+mul+max + ACT Exp.
PATHA_COUNT = 4


def build_nc(n=N):
    from concourse import bacc
    nc = bacc.Bacc("TRN2", target_bir_lowering=False, debug=False,
                   enable_asserts=False, num_devices=N_CORES)

    x_d = nc.declare_dram_parameter("x", [n, F], F32, isOutput=False)
    W_d, b_d, a_d = {}, {}, {}
    for l in (0, 1):
        for h in (0, 1):
            W_d[l, h] = nc.declare_dram_parameter(f"W_{l}_{h}", [F, D], F32, isOutput=False)
            b_d[l, h] = nc.declare_dram_parameter(f"b_{l}_{h}", [D], F32, isOutput=False)
            a_d[l, h] = nc.declare_dram_parameter(f"a_{l}_{h}", [2 * D, 1], F32, isOutput=False)
    out_d = nc.declare_dram_parameter("out", [n, F], F32, isOutput=True)

    NJ = n // P          # node chunks of 128 (partition dim of score tiles)
    IB = min(512, n)     # i-block width (one PSUM bank)
    NI = n // IB
    HW = 2 * IB          # half width (score tile free dim)
    NHALF = n // HW
    KH = HW // IB

    with tile.TileContext(nc) as tc, ExitStack() as ctx:
        const = ctx.enter_context(tc.tile_pool(name="const", bufs=1))
        persist = ctx.enter_context(tc.tile_pool(name="persist", bufs=1))
        headp = ctx.enter_context(tc.tile_pool(name="headp", bufs=2))
        ztp = ctx.enter_context(tc.tile_pool(name="ztp", bufs=8))
        zap = ctx.enter_context(tc.tile_pool(name="zap", bufs=4))
        lrp = ctx.enter_context(tc.tile_pool(name="lrp", bufs=4))
        up = ctx.enter_context(tc.tile_pool(name="up", bufs=16))
        epp = ctx.enter_context(tc.tile_pool(name="epp", bufs=2))
        smallp = ctx.enter_context(tc.tile_pool(name="smallp", bufs=4))
        ps_prep = ctx.enter_context(tc.tile_pool(name="ps_prep", bufs=2, space="PSUM"))
        ps_main = ctx.enter_context(tc.tile_pool(name="ps_main", bufs=2, space="PSUM"))
        ps_z = ctx.enter_context(tc.tile_pool(name="ps_z", bufs=1, space="PSUM"))

        # ---- constants ----
        I128 = const.tile([P, P], F32, tag="I128", name="I128")
        make_identity(nc, I128[:])
        I128b = const.tile([P, P], BF16, tag="I128b", name="I128b")
        nc.vector.tensor_copy(I128b[:], I128[:])
        ones_col_f = const.tile([P, 1], F32, tag="ones_col_f", name="ones_col_f")
        nc.vector.memset(ones_col_f[:], 1.0)
        # ones4[0]: cols [1,1,0,0] -> head0 denominator in rows 0-1;
        # ones4[1]: cols [0,0,1,1] -> head1 denominator in rows 2-3.
        ones4 = []
        for hi in range(2):
            o4f = const.tile([P, 4], F32, tag=f"o4f{hi}", name=f"o4f{hi}")
            nc.vector.memset(o4f[:], 0.0)
            nc.vector.memset(o4f[:, 2 * hi:2 * hi + 2], 1.0)
            o4 = const.tile([P, 4], BF16, tag=f"o4{hi}", name=f"o4{hi}")
            nc.vector.tensor_copy(o4[:], o4f[:])
            ones4.append(o4)
        # sel4[hi]: [4, P] selector stationary - row 2*hi is ones, other rows
        # zero - so matmul(sel4[hi], recip[4, IB]) broadcasts head hi's
        # reciprocal row across all 128 partitions with every AP at partition 0.
        sel4 = []
        for hi in range(2):
            s4f = const.tile([4, P], F32, tag=f"s4f{hi}", name=f"s4f{hi}")
            nc.gpsimd.memset(s4f[:], 0.0)
            # keep 0 where p != 2*hi, fill 1.0 on row p == 2*hi
            nc.gpsimd.affine_select(
                out=s4f[:], in_=s4f[:], compare_op=AluOpType.not_equal,
                fill=1.0, base=-2 * hi, pattern=[[0, P]], channel_multiplier=1)
            s4 = const.tile([4, P], BF16, tag=f"s4{hi}", name=f"s4{hi}")
            nc.vector.tensor_copy(s4[:], s4f[:])
            sel4.append(s4)

        # ---- parameters ----
        Wt, bt, Alt, art = {}, {}, {}, {}
        for l in (0, 1):
            for h in (0, 1):
                Wt[l, h] = []
                for c in range(2):
                    wf = smallp.tile([P, D], F32, tag="wload", name="wload")
                    nc.sync.dma_start(out=wf[:], in_=W_d[l, h][c * P:(c + 1) * P, :])
                    w = const.tile([P, D], F32R, tag=f"W{l}{h}{c}", name=f"W{l}{h}{c}")
                    nc.vector.tensor_copy(w[:], wf[:])
                    Wt[l, h].append(w)
                b = const.tile([P, 1], F32, tag=f"b{l}{h}", name=f"b{l}{h}")
                nc.sync.dma_start(
                    out=b[:], in_=b_d[l, h][:].rearrange("(p o) -> p o", o=1))
                bt[l, h] = b
                alf = smallp.tile([P, 1], F32, tag="alload", name="alload")
                nc.sync.dma_start(out=alf[:], in_=a_d[l, h][0:P, 0:1])
                Al = const.tile([P, P], BF16, tag=f"Al{l}{h}", name=f"Al{l}{h}")
                nc.vector.tensor_copy(Al[:], alf[:].to_broadcast([P, P]))
                Alt[l, h] = Al
                arf = smallp.tile([P, 1], F32, tag="arload", name="arload")
                nc.sync.dma_start(out=arf[:], in_=a_d[l, h][P:2 * P, 0:1])
                ar2 = const.tile([P, 2], BF16, tag=f"ar{l}{h}", name=f"ar{l}{h}")
                nc.vector.tensor_copy(ar2[:], arf[:].to_broadcast([P, 2]))
                art[l, h] = ar2

        # ---- load x and transpose to xT [2 x (P, n)] (f32r: feeds hT-mm) ----
        xT = [persist.tile([P, n], F32R, tag=f"xT{f}", name=f"xT{f}") for f in range(2)]
        for c in range(NJ):
            xc = smallp.tile([P, F], F32, tag="xload", name="xload")
            nc.sync.dma_start(out=xc[:], in_=x_d[c * P:(c + 1) * P, :])
            for f in range(2):
                tp = ps_prep.tile([P, IB], F32, tag="prep", name="prep")
                nc.tensor.transpose(tp[:, 0:P], xc[:, f * P:(f + 1) * P], I128[:])
                if (c + f) % 2 == 0:
                    nc.vector.tensor_copy(xT[f][:, c * P:(c + 1) * P], tp[:, 0:P])
                else:
                    nc.scalar.activation(xT[f][:, c * P:(c + 1) * P], tp[:, 0:P], AF.Copy)

        X1T = [persist.tile([P, n], F32R, tag=f"X1T{f}", name=f"X1T{f}") for f in range(2)]
        X2T = [persist.tile([P, n], F32, tag=f"X2T{f}", name=f"X2T{f}") for f in range(2)]

        def gat_layer(XT, pars, OUTS):
            # ---- per-head prep, issued interleaved ----
            hd = [dict(), dict()]
            for hi in range(2):
                hd[hi]["hT"] = headp.tile([P, n], F32R, tag="hT", name=f"hT{hi}")
                hd[hi]["hTf"] = hd[hi]["hT"][:].bitcast(F32)
                hd[hi]["hTb"] = headp.tile([P, n], BF16, tag="hTb", name=f"hTb{hi}")
                hd[hi]["hlb"] = headp.tile([P, n], BF16, tag="hlb", name=f"hlb{hi}")
                hd[hi]["hrcf"] = headp.tile([P, NJ], F32, tag="hrcf", name=f"hrcf{hi}")
                hd[hi]["hs"] = []
            for ib in range(NI):
                sl = slice(ib * IB, (ib + 1) * IB)
                for hi, (Wc, b, Al, ar2) in enumerate(pars):
                    ps = ps_prep.tile([P, IB], F32, tag="prep", name="prep")
                    nc.tensor.matmul(ps[:], Wc[0][:], XT[0][:, sl], start=True, stop=False)
                    nc.tensor.matmul(ps[:], Wc[1][:], XT[1][:, sl], start=False, stop=True)
                    nc.vector.tensor_scalar_add(hd[hi]["hT"][:, sl], ps[:], b[:])
            for hi in range(2):
                nc.vector.tensor_copy(hd[hi]["hTb"][:], hd[hi]["hTf"][:])
            # h chunks [node_part, d_free] bf16 via PE transpose
            for jc in range(NJ):
                for hi in range(2):
                    tp = ps_prep.tile([P, IB], BF16, tag="prep", name="prep")
                    nc.tensor.transpose(tp[:, 0:P], hd[hi]["hTb"][:, jc * P:(jc + 1) * P], I128b[:])
                    hj = headp.tile([P, P], BF16, tag=f"h{jc}", name=f"h{jc}_{hi}")
                    if (jc + hi) % 2 == 0:
                        nc.vector.tensor_copy(hj[:], tp[:, 0:P])
                    else:
                        nc.scalar.activation(hj[:], tp[:, 0:P], AF.Copy)
                    hd[hi]["hs"].append(hj)
            # hl broadcast across partitions + hr column layout
            for ib in range(NI):
                sl = slice(ib * IB, (ib + 1) * IB)
                for hi, (Wc, b, Al, ar2) in enumerate(pars):
                    ps = ps_prep.tile([P, IB], F32, tag="prep", name="prep")
                    nc.tensor.matmul(ps[:], Al[:], hd[hi]["hTb"][:, sl], start=True, stop=True)
                    nc.vector.tensor_copy(hd[hi]["hlb"][:, sl], ps[:])
            for hi, (Wc, b, Al, ar2) in enumerate(pars):
                psr = ps_prep.tile([P, IB], F32, tag="prep", name="prep")
                for jc in range(NJ):
                    nc.tensor.matmul(psr[:, 2 * jc:2 * jc + 2],
                                     hd[hi]["hTb"][:, jc * P:(jc + 1) * P],
                                     ar2[:], start=True, stop=True)
                pair = psr[:, 0:2 * NJ].rearrange("p (c t) -> p c t", t=2)
                nc.vector.tensor_copy(hd[hi]["hrcf"][:], pair[:, :, 0])

            # ---- score loops, the two heads interleaved per tile ----
            # The previous half's epilogue is deferred and emitted a few
            # tiles into the next half's loop: the next half's DVE/ACT
            # score-gen fills the queue ahead of the epilogue ops (which
            # block on the PE finishing the previous half's matmul tail),
            # avoiding head-of-line stalls. The deferred tiles' matmuls are
            # flushed AFTER the epilogue so its rb matmuls stay ahead of
            # them in the PE queue (else PE deadlocks on the bank ring).
            def make_epilogue(half, zp, oaccs, hTfs):
                def emit():
                    for k in range(KH):
                        ib = half * KH + k
                        isl = slice(ib * IB, (ib + 1) * IB)
                        recip_f = smallp.tile([4, IB], F32, tag="recip_f",
                                              name="recip_f", bufs=2)
                        nc.vector.reciprocal_approx_fast(recip_f[:], zp[k][:])
                        recip = smallp.tile([4, IB], BF16, tag="recip",
                                            name="recip", bufs=2)
                        nc.vector.tensor_copy(recip[:], recip_f[:])
                        for hi in range(2):
                            rb = ps_prep.tile([P, IB], F32, tag="prep", name="prep")
                            nc.tensor.matmul(rb[:], sel4[hi][:], recip[:],
                                             start=True, stop=True)
                            rbs = epp.tile([P, IB], F32, tag="rbs", name="rbs")
                            nc.vector.tensor_copy(rbs[:], rb[:])
                            v = epp.tile([P, IB], F32, tag="v", name="v")
                            nc.vector.tensor_tensor(v[:], oaccs[hi][k][:], rbs[:],
                                                    AluOpType.mult)
                            v2 = epp.tile([P, IB], F32, tag="v2", name="v2")
                            nc.vector.tensor_tensor(v2[:], v[:], hTfs[hi][:, isl],
                                                    AluOpType.add)
                            # elu(v2) = relu(v2) + exp(-relu(-v2)) - 1
                            r1 = epp.tile([P, IB], F32, tag="r1", name="r1")
                            nc.scalar.activation(r1[:], v2[:], AF.Relu, scale=-1.0)
                            r3 = epp.tile([P, IB], F32, tag="r3", name="r3")
                            nc.vector.tensor_scalar(r3[:], v2[:], 0.0, None,
                                                    AluOpType.max)
                            r2 = epp.tile([P, IB], F32, tag="r2", name="r2")
                            nc.scalar.activation(r2[:], r1[:], AF.Exp, scale=-1.0)
                            nc.vector.scalar_tensor_tensor(
                                OUTS[hi][:, isl], in0=r2[:], scalar=-1.0, in1=r3[:],
                                op0=AluOpType.add, op1=AluOpType.add)
                return emit

            PIPE_TILES = 6
            pending_epi = None
            for half in range(NHALF):
                hsl = slice(half * HW, (half + 1) * HW)
                for hi in range(2):
                    hd[hi]["oacc"] = [
                        ps_main.tile([P, IB], F32, tag=f"oacc{k}", name=f"oacc{k}_{hi}")
                        for k in range(KH)]
                # both heads' denominators share one [4, IB] tile per k
                zp = [ps_z.tile([4, IB], F32, tag=f"zp{k}", name=f"zp{k}")
                      for k in range(KH)]
                dlo, dhi = half * (NJ // NHALF), (half + 1) * (NJ // NHALF)
                diag_jcs = [jc for jc in range(NJ) if dlo <= jc < dhi]
                nond_jcs = [jc for jc in range(NJ) if not (dlo <= jc < dhi)]
                # spread the ACT-only tiles uniformly among the DVE-path
                # tiles (clustering them starves DVE/PE at the boundary)
                patha = set(nond_jcs[::2][:PATHA_COUNT])
                if len(patha) < PATHA_COUNT:
                    patha |= set(nond_jcs[1::2][:PATHA_COUNT - len(patha)])
                # alternate diagonal (always DVE-path) and non-diagonal jc
                jc_order = []
                for a, bjc in zip(diag_jcs, nond_jcs):
                    jc_order += [a, bjc]
                deferred = []
                npipe = PIPE_TILES if pending_epi is not None else 0
                tile_idx = 0
                last_jc = jc_order[-1]
                first_jc = jc_order[0]
                for jc in jc_order:
                    diag = dlo <= jc < dhi
                    for hi in range(2):
                        H = hd[hi]
                        u = up.tile([P, HW], BF16, tag="u", name="u")
                        if jc in patha:
                            lr = lrp.tile([P, HW], BF16, tag="lr", name="lr")
                            nc.scalar.activation(lr[:], H["hlb"][:, hsl], AF.Prelu,
                                                 bias=H["hrcf"][:, jc:jc + 1], alpha=ALPHA)
                            nc.scalar.activation(u[:], lr[:], AF.Exp)
                        else:
                            zt = ztp.tile([P, HW], BF16, tag="zt", name="zt")
                            nc.vector.tensor_scalar_add(zt[:], H["hlb"][:, hsl],
                                                        H["hrcf"][:, jc:jc + 1])
                            za = zap.tile([P, HW], BF16, tag="za", name="za")
                            nc.vector.tensor_scalar_mul(za[:], zt[:], ALPHA)
                            nc.vector.tensor_tensor(zt[:], zt[:], za[:],
                                                    AluOpType.max)
                            if diag:
                                off = jc * P - half * HW
                                nc.gpsimd.affine_select(
                                    out=zt[:, off:off + P], in_=zt[:, off:off + P],
                                    compare_op=AluOpType.not_equal,
                                    fill=NEG, base=0, pattern=[[-1, P]],
                                    channel_multiplier=1)
                            nc.scalar.activation(u[:], zt[:], AF.Exp)

                        def mms(H=H, u=u, jc=jc, hi=hi):
                            for k in range(KH):
                                nc.tensor.matmul(H["oacc"][k][:], H["hs"][jc][:],
                                                 u[:, k * IB:(k + 1) * IB],
                                                 start=(jc == first_jc),
                                                 stop=(jc == last_jc))
                            for k in range(KH):
                                nc.tensor.matmul(zp[k][:], ones4[hi][:],
                                                 u[:, k * IB:(k + 1) * IB],
                                                 start=(jc == first_jc and hi == 0),
                                                 stop=(jc == last_jc and hi == 1))
                        if tile_idx < npipe:
                            deferred.append(mms)
                        else:
                            if tile_idx == npipe and pending_epi is not None:
                                pending_epi()
                                pending_epi = None
                                for m in deferred:
                                    m()
                                deferred = []
                            mms()
                        tile_idx += 1
                pending_epi = make_epilogue(
                    half, zp, [hd[hi]["oacc"] for hi in range(2)],
                    [hd[hi]["hTf"] for hi in range(2)])
            pending_epi()

        gat_layer(xT, [(Wt[0, 0], bt[0, 0], Alt[0, 0], art[0, 0]),
                       (Wt[0, 1], bt[0, 1], Alt[0, 1], art[0, 1])], X1T)
        gat_layer(X1T, [(Wt[1, 0], bt[1, 0], Alt[1, 0], art[1, 0]),
                        (Wt[1, 1], bt[1, 1], Alt[1, 1], art[1, 1])], X2T)

        # ---- transpose X2T back and store ----
        for c in range(NJ):
            ob = smallp.tile([P, F], F32, tag="ob", name="ob")
            for f in range(2):
                tp = ps_prep.tile([P, IB], F32, tag="prep", name="prep")
                nc.tensor.transpose(tp[:, 0:P], X2T[f][:, c * P:(c + 1) * P], I128[:])
                if (c + f) % 2 == 0:
                    nc.vector.tensor_copy(ob[:, f * P:(f + 1) * P], tp[:, 0:P])
                else:
                    nc.scalar.activation(ob[:, f * P:(f + 1) * P], tp[:, 0:P], AF.Copy)
            nc.sync.dma_start(out=out_d[c * P:(c + 1) * P, :], in_=ob[:])

    nc.compile()
    return nc


_CACHE = {}
LAST_RESULTS = None


def kernel(**inputs):
    global LAST_RESULTS
    from concourse.bass_utils import run_bass_kernel_spmd

    x = np.ascontiguousarray(np.asarray(inputs["x"], dtype=np.float32))
    B = x.shape[0]
    assert B == N_CORES and x.shape[1] == N and x.shape[2] == F

    if "nc" not in _CACHE:
        _CACHE["nc"] = build_nc()
    nc = _CACHE["nc"]

    base = {}
    for l in (0, 1):
        for h in (0, 1):
            base[f"W_{l}_{h}"] = np.ascontiguousarray(
                np.asarray(inputs[f"W_{l}_{h}"], dtype=np.float32))
            base[f"b_{l}_{h}"] = np.ascontiguousarray(
                np.asarray(inputs[f"b_{l}_{h}"], dtype=np.float32))
            base[f"a_{l}_{h}"] = np.ascontiguousarray(
                np.asarray(inputs[f"a_{l}_{h}"], dtype=np.float32))

    in_maps = [dict(base, x=np.ascontiguousarray(x[i])) for i in range(B)]
    res = run_bass_kernel_spmd(nc, in_maps, list(range(N_CORES)),
                               trace=bool(os.environ.get("BASS_TRACE")))
    LAST_RESULTS = res
    out = np.stack([res.results[i]["out"] for i in range(B)], axis=0)
    return out.astype(np.float32)
